# revision 1
# baseline (speedup 1.0000x reference)
"""MoE BaseLayer kernel for Trainium2 (8 NeuronCores, expert parallelism).

Strategy (per the expert-parallelism sharding hint):
  * Host computes token->expert assignment (scores = x @ centroids.T, argmax)
    -- this IS the shard function: tokens are dispatched to the core owning
    their expert (the host-side equivalent of the All2All in the original),
    and the gate alpha = sigmoid(score of the assigned expert) falls out of
    the same routing scores.
  * Core e holds expert e's weights only and runs the BaseSublayer
    (LayerNorm -> FF1 -> ReLU -> FF2 -> residual) + alpha blend for its
    routed tokens. LayerNorm's affine (ln_g, ln_b) is folded into W1/b1 on
    the host (exact reparameterization): relu(((x-mu)*rs*g + b) @ W1 + b1)
    == relu((x-mu)*rs @ (g*W1) + (b@W1 + b1)).
  * Host scatters per-core outputs back to original token order (combine).

Device kernel (per core, C padded routed tokens), tuned from traces:
  * weight DMAs as 1MB 3D-AP transfers in FF consumption order; xs granular
  * PE warm-up matmul spin releases the HAM clock throttle early
  * LayerNorm stats on DVE, rsqrt/normalize split DVE/ACT, PE transpose to
    xhat^T; FF1 (w1 stationary) -> H^T F-major; ReLU+bias on ACT; FF2
    (h stationary, w2 moving) software-pipelined one F-tile behind FF1
  * b2 is seeded into the FF2 accumulators via a K=1 ones-row matmul
  * blend y = x + alpha*(yacc) via ACT scale-copy + DVE residual add
  * all matmuls in float32r (TRN2 fast-FP32, 4x fp32 rate; producers of
    matmul operands must write f32r-rounded outputs)
"""

import numpy as np

E, D, F = 8, 512, 2048
LN_EPS = 1e-5
P = 128

_CACHE = {}


def _build(C, mm_dtype_name="float32r"):
    import concourse.tile as tile
    from concourse import bacc, mybir
    from concourse.masks import make_identity

    f32 = mybir.dt.float32
    mmdt = getattr(mybir.dt, mm_dtype_name)
    ACT = mybir.ActivationFunctionType
    NT = -(-C // P)       # token tiles (last may be partial, C % 64 == 0)
    SZ = [min(P, C - i * P) for i in range(NT)]   # rows per token tile
    KT = D // P           # contraction tiles over D (4)
    FT = F // P           # F tiles (16)
    NG = (NT + 3) // 4    # groups of <=512 tokens (PSUM bank limit)

    HEADW = NT * D + FT + NT          # xs | b1t | alpha, packed per partition
    nc = bacc.Bacc("TRN2", target_bir_lowering=False, num_devices=E)
    head_d = nc.dram_tensor("head", [P, HEADW], f32, kind="ExternalInput")
    wall_d = nc.dram_tensor("wall", [2 * (FT // 4), P, KT * 512], f32,
                            kind="ExternalInput")
    b2r_d = nc.dram_tensor("b2r", [1, D], f32, kind="ExternalInput")
    y_d = nc.dram_tensor("y", [C, D], f32, kind="ExternalOutput")
    scr_d = nc.dram_tensor("scr", [P, 1], f32, kind="ExternalOutput")

    with tile.TileContext(nc) as tc:
        with (
            tc.tile_pool(name="consts", bufs=1) as consts,
            tc.tile_pool(name="wpool", bufs=1) as wpool,
            tc.tile_pool(name="xpool", bufs=1) as xpool,
            tc.tile_pool(name="hpool", bufs=3) as hpool,
            tc.tile_pool(name="spool", bufs=4) as spool,
            tc.tile_pool(name="opool", bufs=3) as opool,
            tc.tile_pool(name="pt", bufs=2 if NT <= 3 else 1, space="PSUM") as pt,
            tc.tile_pool(name="pf1", bufs=2, space="PSUM") as pf1,
            tc.tile_pool(name="pf2", bufs=1, space="PSUM") as pf2,
            tc.tile_pool(name="pwarm", bufs=1, space="PSUM") as pwarm,
        ):
            # ---- constants / warm-up --------------------------------------
            ident = consts.tile([P, P], f32, name="ident", tag="ident")
            make_identity(nc, ident)
            zf = consts.tile([P, 512], f32, name="zf", tag="zf")
            nc.vector.memset(zf, 0.0)
            warmA = consts.tile([P, P], mmdt, name="warmA", tag="warmA")
            nc.vector.tensor_copy(out=warmA, in_=zf[:, :P])
            warmB = consts.tile([P, 512], mmdt, name="warmB", tag="warmB")
            nc.vector.tensor_copy(out=warmB, in_=zf)
            ones1f = consts.tile([1, P], f32, name="ones1f", tag="ones1f")
            nc.vector.memset(ones1f, 1.0)
            ones1 = consts.tile([1, P], mmdt, name="ones1", tag="ones1")
            nc.vector.tensor_copy(out=ones1, in_=ones1f)
            eps_t = consts.tile([P, 1], f32, name="eps_t", tag="eps")
            nc.vector.memset(eps_t, LN_EPS)

            # PE warm-up spin: sustained matmul activity releases the HAM
            # clock throttle (1.2 -> 2.4 GHz) before the real matmuls arrive
            wkeep2 = consts.tile([P, 1], f32, name="wkeep2", tag="wkeep2")
            wps = pwarm.tile([P, 512], f32, name="wps", tag="wps")
            N_WARM = 10
            for wi in range(N_WARM):
                nc.tensor.matmul(
                    wps, warmA, warmB, start=(wi == 0), stop=(wi == N_WARM - 1)
                )

            # ---- input / weight DMA stream (consumption order) ------------
            # host pre-packs everything into SBUF layout so every transfer is
            # fully contiguous in DRAM: "head" = xs|b1t|alpha, "wall" =
            # alternating w1 column-group / w2 quad blocks of 1MB each
            head_t = xpool.tile([P, HEADW], f32, name="head_t", tag="head_t")
            nc.sync.dma_start(out=head_t, in_=head_d[:])
            xs_t = [head_t[: SZ[i], i * D:(i + 1) * D] for i in range(NT)]
            b1T = head_t[:, NT * D:NT * D + FT]
            alT = [
                head_t[: SZ[i], NT * D + FT + i:NT * D + FT + i + 1]
                for i in range(NT)
            ]

            w1g = [None] * (FT // 4)
            w2q = [None] * (FT // 4)

            def load_w1g(g):
                t = wpool.tile([P, KT, 512], mmdt, name=f"w1g{g}", tag=f"w1g{g}")
                nc.sync.dma_start(
                    out=t,
                    in_=wall_d[2 * g].rearrange("p (k f) -> p k f", k=KT).bitcast(mmdt),
                )
                w1g[g] = t

            def load_w2q(g):
                t = wpool.tile([P, 4, D], mmdt, name=f"w2q{g}", tag=f"w2q{g}")
                nc.sync.dma_start(
                    out=t,
                    in_=wall_d[2 * g + 1].rearrange("p (q d) -> p q d", q=4).bitcast(mmdt),
                )
                w2q[g] = t

            load_w1g(0)
            b2r = consts.tile([1, D], mmdt, name="b2r", tag="b2r")
            nc.sync.dma_start(out=b2r, in_=b2r_d[:].bitcast(mmdt))
            load_w2q(0)
            for g in range(1, FT // 4):
                load_w1g(g)
                load_w2q(g)

            # bridge spin: keep the PE active between the first warm-up and
            # the transposes so the HAM clock stays released
            wps2 = pwarm.tile([P, 512], f32, name="wps2", tag="wps")
            N_BRIDGE = 12
            for wi in range(N_BRIDGE):
                nc.tensor.matmul(
                    wps2[:, :256], warmA, warmB[:, :256],
                    start=(wi == 0), stop=(wi == N_BRIDGE - 1),
                )
            nc.scalar.activation(out=wkeep2, in_=wps2[:, 0:1], func=ACT.Copy)

            # ---- per-group compute ----------------------------------------
            for grp in range(NG):
                t0 = grp * 4                      # first token tile of group
                tn = min(4, NT - t0)              # tiles in this group
                Cg = sum(SZ[t0:t0 + tn])
                cols = [sum(SZ[t0:i]) for i in range(t0, t0 + tn)]

                # LayerNorm stats: DVE does bn_stats/aggr/recip/normalize,
                # ACT does the sqrt; transposes on PE as soon as each tile's
                # xhat is ready, psum->sbuf casts alternate DVE/ACT
                mvs, rss = [], []
                for i in range(t0, t0 + tn):
                    sz = SZ[i]
                    stats = spool.tile([P, 6], f32, name="stats", tag="stats")
                    nc.vector.bn_stats(out=stats[:sz], in_=xs_t[i])
                    mv = spool.tile([P, 2], f32, name="mv", tag=f"mv{i - t0}")
                    nc.vector.bn_aggr(out=mv[:sz], in_=stats[:sz])
                    mvs.append(mv)
                for i in range(t0, t0 + tn):
                    sz = SZ[i]
                    rs = spool.tile([P, 1], f32, name="rs", tag=f"rs{i - t0}")
                    nc.scalar.activation(
                        out=rs[:sz], in_=mvs[i - t0][:sz, 1:2],
                        func=ACT.Sqrt, bias=eps_t[:sz], scale=1.0,
                    )
                    rss.append(rs)
                for i in range(t0, t0 + tn):
                    sz = SZ[i]
                    nc.vector.reciprocal(out=rss[i - t0][:sz], in_=rss[i - t0][:sz])

                xlnT = [
                    hpool.tile([P, Cg], mmdt, name=f"xlnT{kt}", tag=f"xlnT{kt}")
                    for kt in range(KT)
                ]
                xlns = []
                for i in range(t0, t0 + tn):
                    sz = SZ[i]
                    xln = spool.tile([P, D], f32, name="xln", tag=f"xln{i - t0}")
                    nc.vector.tensor_scalar(
                        out=xln[:sz], in0=xs_t[i],
                        scalar1=mvs[i - t0][:sz, 0:1], scalar2=rss[i - t0][:sz],
                        op0=mybir.AluOpType.subtract, op1=mybir.AluOpType.mult,
                    )
                    xlns.append(xln)
                for kt in range(KT):
                    for i in range(t0, t0 + tn):
                        sz = SZ[i]
                        col = cols[i - t0]
                        ps = pt.tile([P, P], f32, name="ps_t", tag="ps_t")
                        nc.tensor.transpose(
                            ps[:, :sz], xlns[i - t0][:sz, kt * P:(kt + 1) * P],
                            ident[:sz, :sz],
                        )
                        dst = xlnT[kt][:, col:col + sz]
                        if kt % 2 == 1:
                            nc.scalar.activation(
                                out=dst, in_=ps[:, :sz], func=ACT.Copy,
                            )
                        else:
                            nc.vector.tensor_copy(out=dst, in_=ps[:, :sz])

                # FF2 accumulators, seeded with the broadcast b2 row
                yaccs = [
                    pf2.tile([P, D], f32, name=f"yacc{i - t0}", tag=f"yacc{i - t0}")
                    for i in range(t0, t0 + tn)
                ]
                for i in range(t0, t0 + tn):
                    nc.tensor.matmul(
                        yaccs[i - t0][: SZ[i]], ones1[:, : SZ[i]], b2r,
                        start=True, stop=False,
                    )

                # FF1 + FF2, software-pipelined one F-tile apart
                hs = [None] * FT

                def ff1(ft):
                    acc = pf1.tile([P, Cg], f32, name="acc1", tag="acc1")
                    for kt in range(KT):
                        lhsT = w1g[ft // 4][:, kt, (ft % 4) * P:(ft % 4 + 1) * P]
                        nc.tensor.matmul(
                            acc, lhsT, xlnT[kt][:],
                            start=(kt == 0), stop=(kt == KT - 1),
                        )
                    h = hpool.tile([P, Cg], mmdt, name="h", tag="h")
                    nc.scalar.activation(
                        out=h, in_=acc, func=ACT.Relu,
                        bias=b1T[:, ft:ft + 1], scale=1.0,
                    )
                    hs[ft] = h

                def ff2(ft):
                    for i in range(t0, t0 + tn):
                        col = cols[i - t0]
                        nc.tensor.matmul(
                            yaccs[i - t0][: SZ[i]],
                            hs[ft][:, col:col + SZ[i]],
                            w2q[ft // 4][:, ft % 4, :],
                            start=False, stop=(ft == FT - 1),
                        )

                ff1(0)
                for ft in range(1, FT):
                    ff1(ft)
                    ff2(ft - 1)
                ff2(FT - 1)

                # blend: y = x + alpha * yacc  (b2 already inside yacc)
                for i in range(t0, t0 + tn):
                    sz = SZ[i]
                    yo = opool.tile([P, D], f32, name="yo", tag="yo")
                    nc.scalar.activation(
                        out=yo[:sz], in_=yaccs[i - t0][:sz],
                        func=ACT.Copy, scale=alT[i],
                    )
                    nc.vector.tensor_add(out=yo[:sz], in0=yo[:sz], in1=xs_t[i])
                    nc.sync.dma_start(
                        out=y_d[i * P:i * P + sz, :], in_=yo[:sz]
                    )

            # keep-alive so DCE cannot drop the warm-up chains; rides the
            # gpsimd queue at the very end so it never stalls weight DMAs
            wkeep = consts.tile([P, 1], f32, name="wkeep", tag="wkeep")
            nc.scalar.activation(out=wkeep, in_=wps[:, 0:1], func=ACT.Copy)
            nc.vector.tensor_add(out=wkeep, in0=wkeep, in1=wkeep2)
            nc.gpsimd.dma_start(out=scr_d[:], in_=wkeep)

    nc.compile()
    return nc


def _get_nc(C):
    if C not in _CACHE:
        _CACHE[C] = _build(C)
    return _CACHE[C]


def _route(feats, centroids):
    """Token->expert assignment + gate, computed the same way the reference
    does (jax on CPU) so argmax near-ties resolve identically."""
    try:
        import jax
        import jax.numpy as jnp

        with jax.default_device(jax.devices("cpu")[0]):
            scores = jnp.asarray(feats) @ jnp.asarray(centroids).T
            assign = jnp.argmax(scores, axis=1)
            alpha = jax.nn.sigmoid(
                jnp.take_along_axis(scores, assign[:, None], axis=1)
            )
            return np.asarray(assign), np.asarray(alpha, dtype=np.float32)
    except Exception:
        scores = feats @ centroids.T
        assign = np.argmax(scores, axis=1)
        alpha = 1.0 / (1.0 + np.exp(-scores[np.arange(len(assign)), assign]))
        return assign, alpha[:, None].astype(np.float32)


def prepare(x, centroids, ln_g, ln_b, W1, b1, W2, b2):
    """Shard the full inputs: route tokens to experts, build per-core input
    maps. Returns (C, in_maps, idx, orig_shape)."""
    x = np.asarray(x)
    orig_shape = x.shape
    feats = np.ascontiguousarray(x.reshape(-1, D), dtype=np.float32)
    centroids = np.asarray(centroids, dtype=np.float32)

    assign, alpha = _route(feats, centroids)

    idx = [np.nonzero(assign == e)[0] for e in range(E)]
    max_count = max(len(ix) for ix in idx)
    C = max(256, -(-max_count // 64) * 64)

    W1 = np.asarray(W1, dtype=np.float32)
    W2 = np.asarray(W2, dtype=np.float32)
    b1 = np.asarray(b1, dtype=np.float32)
    b2 = np.asarray(b2, dtype=np.float32)
    ln_g = np.asarray(ln_g, dtype=np.float32)
    ln_b = np.asarray(ln_b, dtype=np.float32)

    NT = -(-C // P)
    FT = F // P
    KT = D // P
    HEADW = NT * D + FT + NT
    in_maps = []
    for e in range(E):
        xs = np.zeros((NT * P, D), dtype=np.float32)
        xs[: len(idx[e])] = feats[idx[e]]
        al = np.zeros((NT * P,), dtype=np.float32)
        al[: len(idx[e])] = alpha[idx[e], 0]
        # fold LN affine into the first FFN layer (exact reparameterization)
        w1_eff = ln_g[e][:, None] * W1[e]
        b1_eff = ln_b[e] @ W1[e] + b1[e]

        head = np.empty((P, HEADW), dtype=np.float32)
        head[:, : NT * D] = (
            xs.reshape(NT, P, D).transpose(1, 0, 2).reshape(P, NT * D)
        )
        head[:, NT * D:NT * D + FT] = b1_eff.reshape(FT, P).T
        head[:, NT * D + FT:] = al.reshape(NT, P).T

        wall = np.empty((2 * (FT // 4), P, KT * 512), dtype=np.float32)
        for g in range(FT // 4):
            wall[2 * g] = (
                w1_eff[:, g * 512:(g + 1) * 512]
                .reshape(KT, P, 512).transpose(1, 0, 2).reshape(P, KT * 512)
            )
            wall[2 * g + 1] = (
                W2[e][4 * g * P:(4 * g + 4) * P, :]
                .reshape(4, P, D).transpose(1, 0, 2).reshape(P, 4 * D)
            )
        in_maps.append(
            dict(
                head=head,
                wall=wall,
                b2r=np.ascontiguousarray(b2[e].reshape(1, D)),
            )
        )
    return C, in_maps, idx, orig_shape


def kernel(x, centroids, ln_g, ln_b, W1, b1, W2, b2):
    from concourse.bass_utils import run_bass_kernel_spmd

    C, in_maps, idx, orig_shape = prepare(
        x, centroids, ln_g, ln_b, W1, b1, W2, b2
    )
    nc = _get_nc(C)
    res = run_bass_kernel_spmd(nc, in_maps, core_ids=list(range(E)))

    T = int(np.prod(orig_shape[:-1]))
    out = np.empty((T, D), dtype=np.float32)
    for e in range(E):
        out[idx[e]] = res.results[e]["y"][: len(idx[e])]
    return out.reshape(orig_shape)



# revision 12
# speedup vs baseline: 1.0216x; 1.0216x over previous
"""MoE BaseLayer kernel for Trainium2 (8 NeuronCores, expert parallelism).

Strategy (per the expert-parallelism sharding hint):
  * Host computes token->expert assignment (scores = x @ centroids.T, argmax)
    -- this IS the shard function: tokens are dispatched to the core owning
    their expert (the host-side equivalent of the All2All in the original),
    and the gate alpha = sigmoid(score of the assigned expert) falls out of
    the same routing scores.
  * Core e holds expert e's weights only and runs the BaseSublayer
    (LayerNorm -> FF1 -> ReLU -> FF2 -> residual) + alpha blend for its
    routed tokens. LayerNorm's affine (ln_g, ln_b) is folded into W1/b1 on
    the host (exact reparameterization): relu(((x-mu)*rs*g + b) @ W1 + b1)
    == relu((x-mu)*rs @ (g*W1) + (b@W1 + b1)).
  * Host scatters per-core outputs back to original token order (combine).

Device kernel (per core, C padded routed tokens), tuned from traces:
  * weight DMAs as 1MB 3D-AP transfers in FF consumption order; xs granular
  * PE warm-up matmul spin releases the HAM clock throttle early
  * LayerNorm stats on DVE, rsqrt/normalize split DVE/ACT, PE transpose to
    xhat^T; FF1 (w1 stationary) -> H^T F-major; ReLU+bias on ACT; FF2
    (h stationary, w2 moving) software-pipelined one F-tile behind FF1
  * b2 is seeded into the FF2 accumulators via a K=1 ones-row matmul
  * blend y = x + alpha*(yacc) via ACT scale-copy + DVE residual add
  * all matmuls in float32r (TRN2 fast-FP32, 4x fp32 rate; producers of
    matmul operands must write f32r-rounded outputs)
"""

import numpy as np
import ml_dtypes

BF16 = ml_dtypes.bfloat16

E, D, F = 8, 512, 2048
LN_EPS = 1e-5
P = 128

_CACHE = {}


def _build(C, mm_dtype_name="bfloat16"):
    import concourse.tile as tile
    from concourse import bacc, mybir
    from concourse.masks import make_identity

    f32 = mybir.dt.float32
    mmdt = getattr(mybir.dt, mm_dtype_name)
    ACT = mybir.ActivationFunctionType
    NT = -(-C // P)       # token tiles (last may be partial, C % 64 == 0)
    SZ = [min(P, C - i * P) for i in range(NT)]   # rows per token tile
    KT = D // P           # contraction tiles over D (4)
    FT = F // P           # F tiles (16)
    NG = (NT + 3) // 4    # groups of <=512 tokens (PSUM bank limit)

    HEADW = NT * D + FT + NT          # xs | b1t | alpha, packed per partition
    nc = bacc.Bacc("TRN2", target_bir_lowering=False, num_devices=E)
    head_d = nc.dram_tensor("head", [P, HEADW], f32, kind="ExternalInput")
    wall_d = nc.dram_tensor("wall", [2 * (FT // 4), P, KT * 512], mmdt,
                            kind="ExternalInput")
    b2r_d = nc.dram_tensor("b2r", [1, D], mmdt, kind="ExternalInput")
    y_d = nc.dram_tensor("y", [C, D], f32, kind="ExternalOutput")
    scr_d = nc.dram_tensor("scr", [P, 1], f32, kind="ExternalOutput")

    with tile.TileContext(nc) as tc:
        with (
            tc.tile_pool(name="consts", bufs=1) as consts,
            tc.tile_pool(name="wpool", bufs=1) as wpool,
            tc.tile_pool(name="xpool", bufs=1) as xpool,
            tc.tile_pool(name="hpool", bufs=3) as hpool,
            tc.tile_pool(name="spool", bufs=4) as spool,
            tc.tile_pool(name="opool", bufs=3) as opool,
            tc.tile_pool(name="pt", bufs=2 if NT <= 3 else 1, space="PSUM") as pt,
            tc.tile_pool(name="pf1", bufs=2, space="PSUM") as pf1,
            tc.tile_pool(name="pf2", bufs=1, space="PSUM") as pf2,
            tc.tile_pool(name="pwarm", bufs=1, space="PSUM") as pwarm,
        ):
            # ---- constants / warm-up --------------------------------------
            ident = consts.tile([P, P], f32, name="ident", tag="ident")
            make_identity(nc, ident)
            identm = consts.tile([P, P], mmdt, name="identm", tag="identm")
            nc.vector.tensor_copy(out=identm, in_=ident)
            zf = consts.tile([P, 512], f32, name="zf", tag="zf")
            nc.vector.memset(zf, 0.0)
            warmA = consts.tile([P, P], mmdt, name="warmA", tag="warmA")
            nc.vector.tensor_copy(out=warmA, in_=zf[:, :P])
            warmB = consts.tile([P, 512], mmdt, name="warmB", tag="warmB")
            nc.vector.tensor_copy(out=warmB, in_=zf)
            ones1f = consts.tile([1, P], f32, name="ones1f", tag="ones1f")
            nc.vector.memset(ones1f, 1.0)
            ones1 = consts.tile([1, P], mmdt, name="ones1", tag="ones1")
            nc.vector.tensor_copy(out=ones1, in_=ones1f)
            eps_t = consts.tile([P, 1], f32, name="eps_t", tag="eps")
            nc.vector.memset(eps_t, LN_EPS)

            # PE warm-up spin: sustained matmul activity releases the HAM
            # clock throttle (1.2 -> 2.4 GHz) before the real matmuls arrive
            wkeep2 = consts.tile([P, 1], f32, name="wkeep2", tag="wkeep2")
            wps = pwarm.tile([P, 512], f32, name="wps", tag="wps")
            N_WARM = 10
            for wi in range(N_WARM):
                nc.tensor.matmul(
                    wps, warmA, warmB, start=(wi == 0), stop=(wi == N_WARM - 1)
                )

            # ---- input / weight DMA stream (consumption order) ------------
            # host pre-packs everything into SBUF layout so every transfer is
            # fully contiguous in DRAM: "head" = xs|b1t|alpha, "wall" =
            # alternating w1 column-group / w2 quad blocks of 1MB each
            head_t = xpool.tile([P, HEADW], f32, name="head_t", tag="head_t")
            nc.sync.dma_start(out=head_t, in_=head_d[:])
            xs_t = [head_t[: SZ[i], i * D:(i + 1) * D] for i in range(NT)]
            b1T = head_t[:, NT * D:NT * D + FT]
            alT = [
                head_t[: SZ[i], NT * D + FT + i:NT * D + FT + i + 1]
                for i in range(NT)
            ]

            w1g = [None] * (FT // 4)
            w2q = [None] * (FT // 4)

            def load_w1g(g):
                t = wpool.tile([P, KT, 512], mmdt, name=f"w1g{g}", tag=f"w1g{g}")
                nc.sync.dma_start(
                    out=t,
                    in_=wall_d[2 * g].rearrange("p (k f) -> p k f", k=KT),
                )
                w1g[g] = t

            def load_w2q(g):
                t = wpool.tile([P, 4, D], mmdt, name=f"w2q{g}", tag=f"w2q{g}")
                nc.sync.dma_start(
                    out=t,
                    in_=wall_d[2 * g + 1].rearrange("p (q d) -> p q d", q=4),
                )
                w2q[g] = t

            load_w1g(0)
            b2r = consts.tile([1, D], mmdt, name="b2r", tag="b2r")
            nc.sync.dma_start(out=b2r, in_=b2r_d[:])
            load_w2q(0)
            for g in range(1, FT // 4):
                load_w1g(g)
                load_w2q(g)

            # bridge spin: keep the PE active between the first warm-up and
            # the transposes so the HAM clock stays released
            wps2 = pwarm.tile([P, 512], f32, name="wps2", tag="wps")
            N_BRIDGE = 12
            for wi in range(N_BRIDGE):
                nc.tensor.matmul(
                    wps2[:, :256], warmA, warmB[:, :256],
                    start=(wi == 0), stop=(wi == N_BRIDGE - 1),
                )
            nc.scalar.activation(out=wkeep2, in_=wps2[:, 0:1], func=ACT.Copy)

            # ---- per-group compute ----------------------------------------
            for grp in range(NG):
                t0 = grp * 4                      # first token tile of group
                tn = min(4, NT - t0)              # tiles in this group
                Cg = sum(SZ[t0:t0 + tn])
                cols = [sum(SZ[t0:i]) for i in range(t0, t0 + tn)]

                # LayerNorm stats: DVE does bn_stats/aggr/recip/normalize,
                # ACT does the sqrt; transposes on PE as soon as each tile's
                # xhat is ready, psum->sbuf casts alternate DVE/ACT
                mvs, rss = [], []
                for i in range(t0, t0 + tn):
                    sz = SZ[i]
                    stats = spool.tile([P, 6], f32, name="stats", tag="stats")
                    nc.vector.bn_stats(out=stats[:sz], in_=xs_t[i])
                    mv = spool.tile([P, 2], f32, name="mv", tag=f"mv{i - t0}")
                    nc.vector.bn_aggr(out=mv[:sz], in_=stats[:sz])
                    mvs.append(mv)
                for i in range(t0, t0 + tn):
                    sz = SZ[i]
                    rs = spool.tile([P, 1], f32, name="rs", tag=f"rs{i - t0}")
                    nc.scalar.activation(
                        out=rs[:sz], in_=mvs[i - t0][:sz, 1:2],
                        func=ACT.Sqrt, bias=eps_t[:sz], scale=1.0,
                    )
                    rss.append(rs)
                for i in range(t0, t0 + tn):
                    sz = SZ[i]
                    nc.vector.reciprocal(out=rss[i - t0][:sz], in_=rss[i - t0][:sz])

                xlnT = [
                    hpool.tile([P, Cg], mmdt, name=f"xlnT{kt}", tag=f"xlnT{kt}")
                    for kt in range(KT)
                ]
                xlns = []
                for i in range(t0, t0 + tn):
                    sz = SZ[i]
                    xln = spool.tile([P, D], mmdt, name="xln", tag=f"xln{i - t0}")
                    nc.vector.tensor_scalar(
                        out=xln[:sz], in0=xs_t[i],
                        scalar1=mvs[i - t0][:sz, 0:1], scalar2=rss[i - t0][:sz],
                        op0=mybir.AluOpType.subtract, op1=mybir.AluOpType.mult,
                    )
                    xlns.append(xln)
                for kt in range(KT):
                    for i in range(t0, t0 + tn):
                        sz = SZ[i]
                        col = cols[i - t0]
                        ps = pt.tile([P, P], mmdt, name="ps_t", tag="ps_t")
                        nc.tensor.transpose(
                            ps[:, :sz], xlns[i - t0][:sz, kt * P:(kt + 1) * P],
                            identm[:sz, :sz],
                        )
                        dst = xlnT[kt][:, col:col + sz]
                        if kt % 2 == 1:
                            nc.scalar.activation(
                                out=dst, in_=ps[:, :sz], func=ACT.Copy,
                            )
                        else:
                            nc.vector.tensor_copy(out=dst, in_=ps[:, :sz])

                # FF2 accumulators, seeded with the broadcast b2 row
                yaccs = [
                    pf2.tile([P, D], f32, name=f"yacc{i - t0}", tag=f"yacc{i - t0}")
                    for i in range(t0, t0 + tn)
                ]
                for i in range(t0, t0 + tn):
                    nc.tensor.matmul(
                        yaccs[i - t0][: SZ[i]], ones1[:, : SZ[i]], b2r,
                        start=True, stop=False,
                    )

                # FF1 + FF2, software-pipelined one F-tile apart
                hs = [None] * FT

                def ff1(ft):
                    acc = pf1.tile([P, Cg], f32, name="acc1", tag="acc1")
                    for kt in range(KT):
                        lhsT = w1g[ft // 4][:, kt, (ft % 4) * P:(ft % 4 + 1) * P]
                        nc.tensor.matmul(
                            acc, lhsT, xlnT[kt][:],
                            start=(kt == 0), stop=(kt == KT - 1),
                        )
                    h = hpool.tile([P, Cg], mmdt, name="h", tag="h")
                    nc.scalar.activation(
                        out=h, in_=acc, func=ACT.Relu,
                        bias=b1T[:, ft:ft + 1], scale=1.0,
                    )
                    hs[ft] = h

                def ff2(ft):
                    for i in range(t0, t0 + tn):
                        col = cols[i - t0]
                        nc.tensor.matmul(
                            yaccs[i - t0][: SZ[i]],
                            hs[ft][:, col:col + SZ[i]],
                            w2q[ft // 4][:, ft % 4, :],
                            start=False, stop=(ft == FT - 1),
                        )

                ff1(0)
                for ft in range(1, FT):
                    ff1(ft)
                    ff2(ft - 1)
                ff2(FT - 1)

                # blend: y = x + alpha * yacc  (b2 already inside yacc)
                for i in range(t0, t0 + tn):
                    sz = SZ[i]
                    yo = opool.tile([P, D], f32, name="yo", tag="yo")
                    nc.scalar.activation(
                        out=yo[:sz], in_=yaccs[i - t0][:sz],
                        func=ACT.Copy, scale=alT[i],
                    )
                    nc.vector.tensor_add(out=yo[:sz], in0=yo[:sz], in1=xs_t[i])
                    nc.sync.dma_start(
                        out=y_d[i * P:i * P + sz, :], in_=yo[:sz]
                    )

            # keep-alive so DCE cannot drop the warm-up chains; rides the
            # gpsimd queue at the very end so it never stalls weight DMAs
            wkeep = consts.tile([P, 1], f32, name="wkeep", tag="wkeep")
            nc.scalar.activation(out=wkeep, in_=wps[:, 0:1], func=ACT.Copy)
            nc.vector.tensor_add(out=wkeep, in0=wkeep, in1=wkeep2)
            nc.gpsimd.dma_start(out=scr_d[:], in_=wkeep)

    nc.compile()
    return nc


def _get_nc(C):
    if C not in _CACHE:
        _CACHE[C] = _build(C)
    return _CACHE[C]


def _route(feats, centroids):
    """Token->expert assignment + gate, computed the same way the reference
    does (jax on CPU) so argmax near-ties resolve identically."""
    try:
        import jax
        import jax.numpy as jnp

        with jax.default_device(jax.devices("cpu")[0]):
            scores = jnp.asarray(feats) @ jnp.asarray(centroids).T
            assign = jnp.argmax(scores, axis=1)
            alpha = jax.nn.sigmoid(
                jnp.take_along_axis(scores, assign[:, None], axis=1)
            )
            return np.asarray(assign), np.asarray(alpha, dtype=np.float32)
    except Exception:
        scores = feats @ centroids.T
        assign = np.argmax(scores, axis=1)
        alpha = 1.0 / (1.0 + np.exp(-scores[np.arange(len(assign)), assign]))
        return assign, alpha[:, None].astype(np.float32)


def prepare(x, centroids, ln_g, ln_b, W1, b1, W2, b2):
    """Shard the full inputs: route tokens to experts, build per-core input
    maps. Returns (C, in_maps, idx, orig_shape)."""
    x = np.asarray(x)
    orig_shape = x.shape
    feats = np.ascontiguousarray(x.reshape(-1, D), dtype=np.float32)
    centroids = np.asarray(centroids, dtype=np.float32)

    assign, alpha = _route(feats, centroids)

    idx = [np.nonzero(assign == e)[0] for e in range(E)]
    max_count = max(len(ix) for ix in idx)
    C = max(256, -(-max_count // 64) * 64)

    W1 = np.asarray(W1, dtype=np.float32)
    W2 = np.asarray(W2, dtype=np.float32)
    b1 = np.asarray(b1, dtype=np.float32)
    b2 = np.asarray(b2, dtype=np.float32)
    ln_g = np.asarray(ln_g, dtype=np.float32)
    ln_b = np.asarray(ln_b, dtype=np.float32)

    NT = -(-C // P)
    FT = F // P
    KT = D // P
    HEADW = NT * D + FT + NT
    in_maps = []
    for e in range(E):
        xs = np.zeros((NT * P, D), dtype=np.float32)
        xs[: len(idx[e])] = feats[idx[e]]
        al = np.zeros((NT * P,), dtype=np.float32)
        al[: len(idx[e])] = alpha[idx[e], 0]
        # fold LN affine into the first FFN layer (exact reparameterization)
        w1_eff = ln_g[e][:, None] * W1[e]
        b1_eff = ln_b[e] @ W1[e] + b1[e]

        head = np.empty((P, HEADW), dtype=np.float32)
        head[:, : NT * D] = (
            xs.reshape(NT, P, D).transpose(1, 0, 2).reshape(P, NT * D)
        )
        head[:, NT * D:NT * D + FT] = b1_eff.reshape(FT, P).T
        head[:, NT * D + FT:] = al.reshape(NT, P).T

        wall = np.empty((2 * (FT // 4), P, KT * 512), dtype=BF16)
        for g in range(FT // 4):
            wall[2 * g] = (
                w1_eff[:, g * 512:(g + 1) * 512]
                .reshape(KT, P, 512).transpose(1, 0, 2).reshape(P, KT * 512)
            )
            wall[2 * g + 1] = (
                W2[e][4 * g * P:(4 * g + 4) * P, :]
                .reshape(4, P, D).transpose(1, 0, 2).reshape(P, 4 * D)
            )
        in_maps.append(
            dict(
                head=head,
                wall=wall,
                b2r=np.ascontiguousarray(b2[e].reshape(1, D).astype(BF16)),
            )
        )
    return C, in_maps, idx, orig_shape


def kernel(x, centroids, ln_g, ln_b, W1, b1, W2, b2):
    from concourse.bass_utils import run_bass_kernel_spmd

    C, in_maps, idx, orig_shape = prepare(
        x, centroids, ln_g, ln_b, W1, b1, W2, b2
    )
    nc = _get_nc(C)
    res = run_bass_kernel_spmd(nc, in_maps, core_ids=list(range(E)))

    T = int(np.prod(orig_shape[:-1]))
    out = np.empty((T, D), dtype=np.float32)
    for e in range(E):
        out[idx[e]] = res.results[e]["y"][: len(idx[e])]
    return out.reshape(orig_shape)



# revision 17
# speedup vs baseline: 1.4938x; 1.4622x over previous
"""MoE BaseLayer kernel for Trainium2 (8 NeuronCores, expert parallelism).

Strategy (per the expert-parallelism sharding hint):
  * Host computes token->expert assignment (scores = x @ centroids.T, argmax)
    -- this IS the shard function: tokens are dispatched to the core owning
    their expert (the host-side equivalent of the All2All in the original),
    and the gate alpha = sigmoid(score of the assigned expert) falls out of
    the same routing scores.
  * Core e holds expert e's weights only and runs the BaseSublayer
    (LayerNorm -> FF1 -> ReLU -> FF2 -> residual) + alpha blend for its
    routed tokens. LayerNorm's affine (ln_g, ln_b) is folded into W1/b1 on
    the host (exact reparameterization).
  * Host scatters per-core outputs back to original token order (combine).

Device kernel (per core, C padded routed tokens; C = ceil(max_count/8)*8),
v2 tuned from NTFF traces:
  * all matmul operands bf16 (weights cast host-side -> half the DMA bytes;
    bf16 transposes are 1 cyc/row vs 2 for fp32); rel-err ~2e-3, 10x margin
  * no warm-up spin: the PE p-state ramps within the first ~3us of real
    work and the HAM duty-throttle is outside our control either way
  * LN: DVE bn_stats/bn_aggr, ACT Rsqrt(var+eps) (one op, table set 14
    covers Rsqrt+Relu+Copy so only one ACT table load), DVE normalize
    written directly as bf16
  * xln transposed on PE via one [P,512] PSUM tile per kt (3 transposes in,
    one copy out, copies alternate DVE/ACT)
  * FF1 w1-stationary, streams C tokens; ReLU+b1 alternates ACT/DVE
  * FF2 token-streaming: w2 128x128 blocks stationary, h streams ->
    y^T strips [128(D), C] in PSUM (16*4 matmuls of C rows instead of
    16*NT matmuls of 512 rows); b2 folded in as the per-partition bias of
    the PSUM->SBUF copy; y^T transposed back on PE; blend y = x + alpha*yT
    via ACT scale-copy + DVE residual add; y DMA issued from the DVE queue
  * x shipped bf16 (separate dram tensor from the f32 b1/alpha/b2 meta) to
    shorten the critical head DMA; weights streamed in consumption order
"""

import numpy as np
import ml_dtypes

BF16 = ml_dtypes.bfloat16

E, D, F = 8, 512, 2048
LN_EPS = 1e-5
P = 128

_CACHE = {}


def _build(C):
    import concourse.tile as tile
    from concourse import bacc, mybir
    from concourse.masks import make_identity

    f32 = mybir.dt.float32
    bf = mybir.dt.bfloat16
    ACT = mybir.ActivationFunctionType
    NT = -(-C // P)                   # token tiles (last may be partial)
    SZ = [min(P, C - i * P) for i in range(NT)]
    KT = D // P                       # 4
    FT = F // P                       # 16
    assert NT <= 4, "single-group kernel (C <= 512)"
    cols = [i * P for i in range(NT)]
    MW = FT + NT + KT                 # b1T | alpha | b2T

    nc = bacc.Bacc("TRN2", target_bir_lowering=False, num_devices=E)
    hx_d = nc.dram_tensor("hx", [P, NT * D], bf, kind="ExternalInput")
    hm_d = nc.dram_tensor("hm", [P, MW], f32, kind="ExternalInput")
    wall_d = nc.dram_tensor("wall", [2 * (FT // 4), P, KT * 512], bf,
                            kind="ExternalInput")
    y_d = nc.dram_tensor("y", [C, D], f32, kind="ExternalOutput")

    with tile.TileContext(nc) as tc:
        with (
            tc.tile_pool(name="consts", bufs=1) as consts,
            tc.tile_pool(name="wpool", bufs=1) as wpool,
            tc.tile_pool(name="xpool", bufs=1) as xpool,
            tc.tile_pool(name="hpool", bufs=3) as hpool,
            tc.tile_pool(name="spool", bufs=4) as spool,
            tc.tile_pool(name="opool", bufs=3) as opool,
            tc.tile_pool(name="ppt", bufs=2, space="PSUM") as ppt,
            tc.tile_pool(name="pacc", bufs=2, space="PSUM") as pacc,
            tc.tile_pool(name="pyt", bufs=1, space="PSUM") as pyt,
        ):
            identm = consts.tile([P, P], bf, name="identm", tag="identm")
            make_identity(nc, identm)
            eps_t = consts.tile([P, 1], f32, name="eps_t", tag="eps")
            nc.vector.memset(eps_t, LN_EPS)

            # ---- input DMA stream (consumption order, sync queue) ---------
            hx_t = xpool.tile([P, NT, D], bf, name="hx_t", tag="hx_t")
            for i in range(NT):
                nc.sync.dma_start(out=hx_t[:, i], in_=hx_d[:, i * D:(i + 1) * D])
            hm_t = xpool.tile([P, MW], f32, name="hm_t", tag="hm_t")
            nc.sync.dma_start(out=hm_t, in_=hm_d[:])
            xs_t = [hx_t[: SZ[i], i] for i in range(NT)]
            b1T = hm_t[:, 0:FT]
            alT = [hm_t[: SZ[i], FT + i:FT + i + 1] for i in range(NT)]
            b2T = hm_t[:, FT + NT:FT + NT + KT]

            w1g = [None] * (FT // 4)
            w2q = [None] * (FT // 4)

            def load_w1g(g):
                t = wpool.tile([P, KT, 512], bf, name=f"w1g{g}", tag=f"w1g{g}")
                nc.sync.dma_start(
                    out=t, in_=wall_d[2 * g].rearrange("p (k f) -> p k f", k=KT)
                )
                w1g[g] = t

            def load_w2q(g):
                t = wpool.tile([P, 4, D], bf, name=f"w2q{g}", tag=f"w2q{g}")
                nc.sync.dma_start(
                    out=t, in_=wall_d[2 * g + 1].rearrange("p (q d) -> p q d", q=4)
                )
                w2q[g] = t

            for g in range(FT // 4):
                load_w1g(g)
                load_w2q(g)

            # ---- LayerNorm (DVE stats, ACT rsqrt, DVE normalize -> bf16) --
            mvs, rss = [], []
            for i in range(NT):
                sz = SZ[i]
                stats = spool.tile([P, 6], f32, name="stats", tag="stats")
                nc.vector.bn_stats(out=stats[:sz], in_=xs_t[i])
                mv = spool.tile([P, 2], f32, name="mv", tag=f"mv{i}")
                nc.vector.bn_aggr(out=mv[:sz], in_=stats[:sz])
                mvs.append(mv)
            for i in range(NT):
                sz = SZ[i]
                rs = spool.tile([P, 1], f32, name="rs", tag=f"rs{i}")
                nc.scalar.activation(
                    out=rs[:sz], in_=mvs[i][:sz, 1:2],
                    func=ACT.Sqrt, bias=eps_t[:sz], scale=1.0,
                )
                rss.append(rs)
            for i in range(NT):
                nc.vector.reciprocal(out=rss[i][: SZ[i]], in_=rss[i][: SZ[i]])
            xlns = []
            for i in range(NT):
                sz = SZ[i]
                xln = spool.tile([P, D], bf, name="xln", tag=f"xln{i}")
                nc.vector.tensor_scalar(
                    out=xln[:sz], in0=xs_t[i],
                    scalar1=mvs[i][:sz, 0:1], scalar2=rss[i][:sz],
                    op0=mybir.AluOpType.subtract, op1=mybir.AluOpType.mult,
                )
                xlns.append(xln)

            # ---- transpose xln -> xlnT[kt] [P, C] (PE; copies DVE/ACT) ----
            xlnT = [
                hpool.tile([P, C], bf, name=f"xlnT{kt}", tag=f"xlnT{kt}")
                for kt in range(KT)
            ]
            for kt in range(KT):
                ps = ppt.tile([P, 512], bf, name="ps_t", tag="ps_t")
                for i in range(NT):
                    sz = SZ[i]
                    nc.tensor.transpose(
                        ps[:, cols[i]:cols[i] + sz],
                        xlns[i][:sz, kt * P:(kt + 1) * P],
                        identm[:sz, :sz],
                    )
                if kt % 2 == 0:
                    nc.vector.tensor_copy(out=xlnT[kt], in_=ps[:, :C])
                else:
                    nc.scalar.activation(out=xlnT[kt], in_=ps[:, :C], func=ACT.Copy)

            # ---- FF1 + FF2 (token-streaming), pipelined one F-tile apart --
            yT = [
                pyt.tile([P, C], f32, name=f"yt{dt}", tag=f"yt{dt}")
                for dt in range(KT)
            ]
            hs = [None] * FT

            def ff1(ft):
                acc = pacc.tile([P, C], f32, name="acc1", tag="acc1")
                for kt in range(KT):
                    lhsT = w1g[ft // 4][:, kt, (ft % 4) * P:(ft % 4 + 1) * P]
                    nc.tensor.matmul(
                        acc, lhsT, xlnT[kt][:],
                        start=(kt == 0), stop=(kt == KT - 1),
                    )
                h = hpool.tile([P, C], bf, name="h", tag="h")
                if ft % 2 == 0:
                    nc.scalar.activation(
                        out=h, in_=acc, func=ACT.Relu,
                        bias=b1T[:, ft:ft + 1], scale=1.0,
                    )
                else:
                    nc.vector.tensor_scalar(
                        out=h, in0=acc,
                        scalar1=b1T[:, ft:ft + 1], scalar2=0.0,
                        op0=mybir.AluOpType.add, op1=mybir.AluOpType.max,
                    )
                hs[ft] = h

            def ff2(ft):
                for dt in range(KT):
                    lhsT = w2q[ft // 4][:, ft % 4, dt * P:(dt + 1) * P]
                    nc.tensor.matmul(
                        yT[dt], lhsT, hs[ft][:],
                        start=(ft == 0), stop=(ft == FT - 1),
                    )

            ff1(0)
            for ft in range(1, FT):
                ff1(ft)
                ff2(ft - 1)
            ff2(FT - 1)

            # ---- y^T + b2 -> SBUF (b2 is a per-partition bias here) -------
            yTs = []
            for dt in range(KT):
                t = spool.tile([P, C], bf, name=f"yTs{dt}", tag=f"yTs{dt}")
                nc.vector.tensor_scalar(
                    out=t, in0=yT[dt][:, :C],
                    scalar1=b2T[:, dt:dt + 1], scalar2=None,
                    op0=mybir.AluOpType.add,
                )
                yTs.append(t)

            # ---- transpose back, blend y = x + alpha * (ffn + b2), store --
            for i in range(NT):
                sz = SZ[i]
                yb = ppt.tile([P, 512], bf, name="ps_t", tag="ps_t")
                for dt in range(KT):
                    nc.tensor.transpose(
                        yb[:sz, dt * P:(dt + 1) * P],
                        yTs[dt][:, cols[i]:cols[i] + sz],
                        identm,
                    )
                yo = opool.tile([P, D], f32, name="yo", tag="yo")
                nc.scalar.activation(
                    out=yo[:sz], in_=yb[:sz], func=ACT.Copy, scale=alT[i],
                )
                nc.vector.tensor_add(out=yo[:sz], in0=yo[:sz], in1=xs_t[i])
                nc.gpsimd.dma_start(out=y_d[i * P:i * P + sz, :], in_=yo[:sz])

    nc.compile()
    return nc


def _get_nc(C):
    if C not in _CACHE:
        _CACHE[C] = _build(C)
    return _CACHE[C]


def _route(feats, centroids):
    """Token->expert assignment + gate, computed the same way the reference
    does (jax on CPU) so argmax near-ties resolve identically."""
    try:
        import jax
        import jax.numpy as jnp

        with jax.default_device(jax.devices("cpu")[0]):
            scores = jnp.asarray(feats) @ jnp.asarray(centroids).T
            assign = jnp.argmax(scores, axis=1)
            alpha = jax.nn.sigmoid(
                jnp.take_along_axis(scores, assign[:, None], axis=1)
            )
            return np.asarray(assign), np.asarray(alpha, dtype=np.float32)
    except Exception:
        scores = feats @ centroids.T
        assign = np.argmax(scores, axis=1)
        alpha = 1.0 / (1.0 + np.exp(-scores[np.arange(len(assign)), assign]))
        return assign, alpha[:, None].astype(np.float32)


def prepare(x, centroids, ln_g, ln_b, W1, b1, W2, b2):
    """Shard the full inputs: route tokens to experts, build per-core input
    maps. Returns (C, in_maps, idx, orig_shape)."""
    x = np.asarray(x)
    orig_shape = x.shape
    feats = np.ascontiguousarray(x.reshape(-1, D), dtype=np.float32)
    centroids = np.asarray(centroids, dtype=np.float32)

    assign, alpha = _route(feats, centroids)

    idx = [np.nonzero(assign == e)[0] for e in range(E)]
    max_count = max(len(ix) for ix in idx)
    C = max(128, -(-max_count // 8) * 8)

    W1 = np.asarray(W1, dtype=np.float32)
    W2 = np.asarray(W2, dtype=np.float32)
    b1 = np.asarray(b1, dtype=np.float32)
    b2 = np.asarray(b2, dtype=np.float32)
    ln_g = np.asarray(ln_g, dtype=np.float32)
    ln_b = np.asarray(ln_b, dtype=np.float32)

    NT = -(-C // P)
    FT = F // P
    KT = D // P
    MW = FT + NT + KT
    in_maps = []
    for e in range(E):
        xs = np.zeros((NT * P, D), dtype=np.float32)
        xs[: len(idx[e])] = feats[idx[e]]
        al = np.zeros((NT * P,), dtype=np.float32)
        al[: len(idx[e])] = alpha[idx[e], 0]
        # fold LN affine into the first FFN layer (exact reparameterization)
        w1_eff = ln_g[e][:, None] * W1[e]
        b1_eff = ln_b[e] @ W1[e] + b1[e]

        hx = (
            xs.reshape(NT, P, D).transpose(1, 0, 2).reshape(P, NT * D)
        ).astype(BF16)
        hm = np.empty((P, MW), dtype=np.float32)
        hm[:, 0:FT] = b1_eff.reshape(FT, P).T
        hm[:, FT:FT + NT] = al.reshape(NT, P).T
        hm[:, FT + NT:] = b2[e].reshape(KT, P).T

        wall = np.empty((2 * (FT // 4), P, KT * 512), dtype=BF16)
        for g in range(FT // 4):
            wall[2 * g] = (
                w1_eff[:, g * 512:(g + 1) * 512]
                .reshape(KT, P, 512).transpose(1, 0, 2).reshape(P, KT * 512)
            )
            wall[2 * g + 1] = (
                W2[e][4 * g * P:(4 * g + 4) * P, :]
                .reshape(4, P, D).transpose(1, 0, 2).reshape(P, 4 * D)
            )
        in_maps.append(dict(hx=hx, hm=hm, wall=wall))
    return C, in_maps, idx, orig_shape


def kernel(x, centroids, ln_g, ln_b, W1, b1, W2, b2):
    from concourse.bass_utils import run_bass_kernel_spmd

    C, in_maps, idx, orig_shape = prepare(
        x, centroids, ln_g, ln_b, W1, b1, W2, b2
    )
    nc = _get_nc(C)
    res = run_bass_kernel_spmd(nc, in_maps, core_ids=list(range(E)))

    T = int(np.prod(orig_shape[:-1]))
    out = np.empty((T, D), dtype=np.float32)
    for e in range(E):
        out[idx[e]] = res.results[e]["y"][: len(idx[e])]
    return out.reshape(orig_shape)
